# revision 21
# baseline (speedup 1.0000x reference)
"""Trainium2 Bass kernel for nn_Conv2d_lsq_int (LSQ int8-style quantized 3x3 conv).

Full-input contract: kernel(**inputs) takes the complete tensors
(x[16,320,64,64], weight[320,320,3,3], bias[320], scalar step sizes) and
returns the full [16,320,64,64] float32 output.

Distribution: data-parallel over the batch dim — 2 images per NeuronCore on
8 cores; weight/bias replicated. The conv itself, input/weight quantization
and the shift/round/clip epilogue all run on device; the host only shards the
batch, re-lays-out the weight to [k, ci, co] (pure data movement), computes
the 320-element bias requant (DVE has no divide op), and concatenates the
per-core outputs.

Math notes:
 - x_int/w_int are integers in [-127,127] -> exact in bf16; the 3x3 conv is
   computed as 9 shifted matmuls per 128-channel cin chunk accumulating fp32
   in PSUM (every intermediate is an exactly-representable integer, so the
   result matches the reference bit-for-bit regardless of order).
 - round() is fp32 add/subtract of 1.5*2^23 (round-to-nearest-even, identical
   to jnp.round), fused as a single-rounding FMA in ScalarE (Copy,scale,bias).
 - cin = 320 = 2.5 * 128: the 64-wide remainder chunk would waste half the PE
   array, so remainder matmuls for ADJACENT row-tiles are packed into the two
   halves of the array (row tiling): x/w for cin 256:320 are duplicated on
   partitions 64:127 and the paired matmuls run concurrently.
"""

import contextlib
import ctypes
import sys
import types

import numpy as np

import concourse.bass as bass  # noqa: F401
import concourse.tile as tile
from concourse import bacc, mybir
from concourse.bass_utils import run_bass_kernel_spmd

F32 = mybir.dt.float32
BF16 = mybir.dt.bfloat16
OP = mybir.AluOpType
ACTF = mybir.ActivationFunctionType

MAGIC = 12582912.0  # 1.5 * 2**23 : fp32 round-to-nearest-even trick
QMAX = 127.0

B, CIN, COUT, H, W, K = 16, 320, 320, 64, 64, 3
N_CORES = 8
IMGS_PER_CORE = B // N_CORES
HW = H * W
PW = W + 2  # padded width
PH = H + 2
ROWS_PER_TILE = 8  # 8 rows * 64 cols = 512 px per psum tile
SLAB = 16  # x load/quant pipelined in 16-row slabs
CHUNKS = [(0, 128), (128, 128), (256, 64)]  # (start, size) along cin / cout


def _install_axon_ntff_hook():
    """Slim antenv.axon_hooks so trace=True works (and never crashes) here."""
    if "antenv.axon_hooks" in sys.modules:
        return
    hook = None
    try:
        lib = ctypes.CDLL("/opt/axon/libaxon_pjrt.so")
        if hasattr(lib, "axon_start_nrt_profile"):
            lib.axon_start_nrt_profile.argtypes = [
                ctypes.POINTER(ctypes.c_int64),
                ctypes.c_size_t,
            ]
            lib.axon_start_nrt_profile.restype = ctypes.c_int64
            lib.axon_stop_nrt_profile.argtypes = [ctypes.c_char_p]
            lib.axon_stop_nrt_profile.restype = ctypes.c_int64

            @contextlib.contextmanager
            def hook(output_dir, device_ids):  # noqa: F811
                import jax

                jax.devices()
                if device_ids:
                    ids = (ctypes.c_int64 * len(device_ids))(*device_ids)
                    rc = lib.axon_start_nrt_profile(ids, len(device_ids))
                else:
                    rc = lib.axon_start_nrt_profile(None, 0)
                if rc != 0:
                    raise RuntimeError(f"axon_start_nrt_profile rc={rc}")
                try:
                    yield
                finally:
                    n = lib.axon_stop_nrt_profile(str(output_dir).encode())
                    print(f"profile: {n} ntff file(s) -> {output_dir}",
                          file=sys.stderr)
    except OSError:
        pass

    mod = types.ModuleType("antenv.axon_hooks")
    mod.get_axon_ntff_profile_hook = lambda: hook
    mod.set_axon_ntff_profile_hook = lambda h: None
    sys.modules["antenv.axon_hooks"] = mod

    # keep profiling artifacts local (zero-egress container)
    import concourse.bass_utils as bu

    bu.upload_artifacts = lambda tmpdir: "local://" + str(tmpdir)


def bias_int8(b, sb, ss, sx, sw):
    """Host fp32 replica of the reference's bias requant (DVE lacks divide).

    Every op is a single IEEE-754 fp32 operation in the reference's exact
    order, so this is bit-identical to the jax fp32 computation.
    """
    f32 = np.float32
    b = np.asarray(b, np.float32)
    b_deq = np.clip(np.round(b / f32(sb)), -QMAX, QMAX).astype(np.float32) * f32(sb)
    x_scale = f32(1.0) / f32(sx)
    w_scale = f32(1.0) / f32(sw)
    t = ((b_deq * f32(ss)) * x_scale) * w_scale
    return np.clip(np.round(t), -QMAX, QMAX).astype(np.float32)


def prep_weight(w):
    """Host layout prep: [co, ci, kh, kw] -> [ci, k*co] (pure data movement).

    Makes the per-cin-chunk weight DMA fully contiguous per partition."""
    return np.ascontiguousarray(
        np.transpose(np.asarray(w, np.float32), (1, 2, 3, 0))
    ).reshape(CIN, K * K * COUT)


def _build(sx: float, sw: float, sb: float, ss: float):
    """Build the per-core Bass program. Scalars are baked as immediates."""
    nc = bacc.Bacc("TRN2", target_bir_lowering=False, debug=False)

    x_d = nc.dram_tensor("x", [IMGS_PER_CORE, CIN, HW], F32, kind="ExternalInput")
    w_d = nc.dram_tensor("w", [CIN, K * K * COUT], F32, kind="ExternalInput")
    b_d = nc.dram_tensor("b", [COUT], F32, kind="ExternalInput")
    y_d = nc.dram_tensor("y", [IMGS_PER_CORE, COUT, HW], F32, kind="ExternalOutput")

    r_x = float(np.float32(1.0) / np.float32(sx))  # x_scale
    r_w = float(np.float32(1.0) / np.float32(sw))  # w_scale
    ss_f = float(np.float32(ss))

    n_slabs = H // SLAB

    with tile.TileContext(nc) as tc:
        with (
            tc.tile_pool(name="persist", bufs=1) as persist,
            tc.tile_pool(name="wstage", bufs=2) as wstage,
            tc.tile_pool(name="xstage", bufs=4) as xstage,
            tc.tile_pool(name="epi", bufs=4) as epi,
            tc.tile_pool(name="psum", bufs=8, space="PSUM") as psum,
        ):
            # --- padded bf16 image buffers; zero only the border ring ------
            # (interior is fully overwritten by the quant writes; full-buffer
            # gpsimd memsets serialized ~25us at startup)
            xq = {}
            for i in range(IMGS_PER_CORE):
                for c in range(len(CHUNKS)):
                    xq_t = persist.tile(
                        [128, PH * PW], BF16, tag=f"xq{i}_{c}", name=f"xq{i}_{c}"
                    )
                    x3 = xq_t.rearrange("p (r w) -> p r w", r=PH)
                    eng = nc.vector
                    eng.memset(x3[:, 0:1, :], 0.0)      # top pad row
                    eng.memset(x3[:, PH - 1 :, :], 0.0)  # bottom pad row
                    eng.memset(x3[:, 1 : PH - 1, 0:1], 0.0)        # left col
                    eng.memset(x3[:, 1 : PH - 1, PW - 1 :], 0.0)   # right col
                    xq[(i, c)] = x3

            # ---------------- weights: quantize to bf16 [ci, k*cout] -----
            # cin remainder chunk (c=2) is duplicated on partitions 64:127
            # for row-packed matmuls.
            wq = {}

            def emit_w_chunk(c):
                ci0, pc = CHUNKS[c]
                wst = wstage.tile([128, K * K * COUT], F32, tag="wst",
                                  name=f"wst{c}")
                qp = pc  # quantized partition count
                half = 5 * COUT
                for lo, hi in ((0, half), (half, K * K * COUT)):
                    nc.sync.dma_start(
                        wst[:pc, lo:hi], w_d[ci0 : ci0 + pc, lo:hi]
                    )
                    if pc < 128:
                        nc.sync.dma_start(
                            wst[pc : 2 * pc, lo:hi], w_d[ci0 : ci0 + pc, lo:hi]
                        )
                if pc < 128:
                    qp = 2 * pc
                wq[c] = persist.tile(
                    [128, K * K * COUT], BF16, tag=f"wq{c}", name=f"wq{c}"
                )
                # quantize in the same two column halves so ACT and DVE
                # pipeline behind the half-DMAs
                for lo, hi in ((0, half), (half, K * K * COUT)):
                    nc.scalar.activation(
                        wst[:qp, lo:hi], wst[:qp, lo:hi], ACTF.Copy,
                        bias=MAGIC, scale=r_w,
                    )
                    nc.vector.tensor_scalar(
                        wst[:qp, lo:hi], wst[:qp, lo:hi], MAGIC, QMAX,
                        OP.subtract, OP.min,
                    )
                    nc.vector.tensor_scalar(
                        wq[c][:qp, lo:hi], wst[:qp, lo:hi], -QMAX, None, OP.max
                    )

            def emit_x_slab(i, s, only_c=None):
                r0 = s * SLAB
                for c, (ci0, pc) in enumerate(CHUNKS):
                    if only_c is not None and c != only_c:
                        continue
                    st = xstage.tile([128, SLAB * W], F32, tag="xst")
                    nc.sync.dma_start(
                        st[:pc, :],
                        x_d[i, ci0 : ci0 + pc, r0 * W : (r0 + SLAB) * W],
                    )
                    qp = pc
                    if pc < 128:
                        nc.sync.dma_start(
                            st[pc : 2 * pc, :],
                            x_d[i, ci0 : ci0 + pc, r0 * W : (r0 + SLAB) * W],
                        )
                        qp = 2 * pc
                    nc.scalar.activation(
                        st[:qp, :], st[:qp, :], ACTF.Copy, bias=MAGIC, scale=r_x
                    )
                    # clip in magic-offset space, single dual-op on DVE
                    nc.vector.tensor_scalar(
                        st[:qp, :], st[:qp, :], MAGIC + QMAX, MAGIC - QMAX,
                        OP.min, OP.max,
                    )
                    # subtract magic + cast to bf16 into the padded buffer;
                    # chunks 0/1 on ACT, chunk 2 on DVE (engine balance)
                    dst = xq[(i, c)][:qp, 1 + r0 : 1 + r0 + SLAB, 1 : W + 1]
                    srcv = st[:qp, :].rearrange("p (r w) -> p r w", r=SLAB)
                    if c < 2:
                        nc.scalar.activation(
                            dst, srcv, ACTF.Copy, bias=-MAGIC, scale=1.0
                        )
                    else:
                        nc.vector.tensor_scalar(
                            dst, srcv, MAGIC, None, OP.subtract
                        )

            # interleave in first-consumption order so the first
            # pair-iter's deps (wq0+x00c0, then wq1+x00c1, ...) finish early
            emit_w_chunk(0)
            emit_x_slab(0, 0, only_c=0)
            emit_w_chunk(1)
            emit_x_slab(0, 0, only_c=1)
            emit_w_chunk(2)
            emit_x_slab(0, 0, only_c=2)
            for s in range(1, n_slabs):
                emit_x_slab(0, s)
            for s in range(n_slabs):
                emit_x_slab(1, s)

            # ------------- b_int8 (host-computed), laid out [128, 4] ------
            # col 3 duplicates col 2 on partitions 64:127 for the
            # column-packed cout-remainder epilogue. First consumer is the
            # first epilogue (~30us in), so these DMAs go after the x slabs.
            bt = persist.tile([128, 4], F32, tag="bias")
            nc.vector.memset(bt[:], 0.0)
            nc.sync.dma_start(
                bt[:, 0:2], b_d[0:256].rearrange("(c p) -> p c", p=128)
            )
            nc.sync.dma_start(
                bt[:64, 2:3], b_d[256:320].rearrange("(p c) -> p c", c=1)
            )
            nc.sync.dma_start(
                bt[64:128, 3:4], b_d[256:320].rearrange("(p c) -> p c", c=1)
            )

            # ---------------- main conv loop ------------------------------
            # row-tiles processed in pairs (A, B) so the 64-wide cin
            # remainder matmuls can be packed into the two array halves.
            n_row_tiles = H // ROWS_PER_TILE

            def emit_epilogue(ps, p0, cot, co0, cs, i, r0):
                # p0: psum/base partition of this output block
                t1 = epi.tile([128, ROWS_PER_TILE * W], F32, tag="t1")
                nc.scalar.activation(
                    t1[p0 : p0 + cs, :],
                    ps[p0 : p0 + cs, :],
                    ACTF.Copy,
                    bias=MAGIC,
                    scale=ss_f,
                )
                nc.vector.tensor_scalar(
                    t1[p0 : p0 + cs, :],
                    t1[p0 : p0 + cs, :],
                    MAGIC,
                    QMAX,
                    OP.subtract,
                    OP.min,
                )
                t2 = epi.tile([128, ROWS_PER_TILE * W], F32, tag="t2")
                bcol = cot if p0 == 0 else 3
                nc.vector.tensor_scalar(
                    t2[p0 : p0 + cs, :],
                    t1[p0 : p0 + cs, :],
                    -QMAX,
                    bt[p0 : p0 + cs, bcol : bcol + 1],
                    OP.max,
                    OP.add,
                )
                nc.vector.tensor_scalar(
                    t2[p0 : p0 + cs, :], t2[p0 : p0 + cs, :], QMAX, -QMAX,
                    OP.min, OP.max,
                )
                nc.sync.dma_start(
                    y_d[i, co0 : co0 + cs, r0 * W : (r0 + ROWS_PER_TILE) * W],
                    t2[p0 : p0 + cs, :],
                )

            for i in range(IMGS_PER_CORE):
                for cot, (co0, cs) in enumerate(CHUNKS):
                    for pt in range(n_row_tiles // 2):
                        rA = (2 * pt) * ROWS_PER_TILE
                        rB = (2 * pt + 1) * ROWS_PER_TILE
                        psA = psum.tile([128, ROWS_PER_TILE * W], F32, tag="ps")
                        psB = psum.tile([128, ROWS_PER_TILE * W], F32, tag="ps")

                        def rhs(c, r0, kh, kw, lo=0, hi=128):
                            return xq[(i, c)][
                                lo:hi, r0 + kh : r0 + kh + ROWS_PER_TILE, kw : kw + W
                            ]

                        def wcol(c, k, lo=0, hi=128):
                            return wq[c][lo:hi, k * COUT + co0 : k * COUT + co0 + cs]

                        if cs == 128:
                            # full 128-deep cin chunks: serial matmuls
                            for ps_, r0 in ((psA, rA), (psB, rB)):
                                first = True
                                for c in (0, 1):
                                    for k in range(K * K):
                                        kh, kw = divmod(k, K)
                                        nc.tensor.matmul(
                                            ps_[:cs, :],
                                            wcol(c, k),
                                            rhs(c, r0, kh, kw),
                                            start=first,
                                            stop=False,
                                        )
                                        first = False
                            # 64-deep cin remainder: row-packed A/B pairs
                            for k in range(K * K):
                                kh, kw = divmod(k, K)
                                last = k == K * K - 1
                                nc.tensor.matmul(
                                    psA[:cs, :],
                                    wcol(2, k, 0, 64),
                                    rhs(2, rA, kh, kw, 0, 64),
                                    start=False,
                                    stop=last,
                                )
                                nc.tensor.matmul(
                                    psB[:cs, :],
                                    wcol(2, k, 64, 128),
                                    rhs(2, rB, kh, kw, 64, 128),
                                    start=False,
                                    stop=last,
                                )
                            emit_epilogue(psA, 0, cot, co0, cs, i, rA)
                            emit_epilogue(psB, 0, cot, co0, cs, i, rB)
                        else:
                            # 64-wide cout remainder: column-pack A/B into
                            # the two column halves of the array.
                            # A -> cols/psum partitions 0:64,
                            # B -> cols/psum partitions 64:128.
                            for c in (0, 1):
                                for k in range(K * K):
                                    kh, kw = divmod(k, K)
                                    first = c == 0 and k == 0
                                    nc.tensor.matmul(
                                        psA[0:cs, :],
                                        wcol(c, k),
                                        rhs(c, rA, kh, kw),
                                        start=first,
                                        stop=False,
                                        tile_position=(0, 0),
                                    )
                                    nc.tensor.matmul(
                                        psB[64 : 64 + cs, :],
                                        wcol(c, k),
                                        rhs(c, rB, kh, kw),
                                        start=first,
                                        stop=False,
                                        tile_position=(0, 64),
                                    )
                            # cin remainder: quadrant-packed
                            for k in range(K * K):
                                kh, kw = divmod(k, K)
                                last = k == K * K - 1
                                nc.tensor.matmul(
                                    psA[0:cs, :],
                                    wcol(2, k, 0, 64),
                                    rhs(2, rA, kh, kw, 0, 64),
                                    start=False,
                                    stop=last,
                                    tile_position=(0, 0),
                                )
                                nc.tensor.matmul(
                                    psB[64 : 64 + cs, :],
                                    wcol(2, k, 64, 128),
                                    rhs(2, rB, kh, kw, 64, 128),
                                    start=False,
                                    stop=last,
                                    tile_position=(64, 64),
                                )
                            emit_epilogue(psA, 0, cot, co0, cs, i, rA)
                            emit_epilogue(psB, 64, cot, co0, cs, i, rB)

    nc.compile()
    return nc


_BUILD_CACHE = {}


def _get_nc(sx, sw, sb, ss):
    key = (sx, sw, sb, ss)
    if key not in _BUILD_CACHE:
        _BUILD_CACHE[key] = _build(sx, sw, sb, ss)
    return _BUILD_CACHE[key]


def _run(x, weight, bias, step_x, step_w, step_b, shift_scale, trace=False):
    _install_axon_ntff_hook()
    x = np.ascontiguousarray(np.asarray(x, dtype=np.float32))
    w = np.asarray(weight, dtype=np.float32)
    b = np.ascontiguousarray(np.asarray(bias, dtype=np.float32))
    sx = float(np.asarray(step_x))
    sw = float(np.asarray(step_w))
    sb = float(np.asarray(step_b))
    ss = float(np.asarray(shift_scale))

    nc = _get_nc(sx, sw, sb, ss)

    w_t = prep_weight(w)
    x_sh = x.reshape(N_CORES, IMGS_PER_CORE, CIN, HW)

    b_i8 = bias_int8(b, sb, ss, sx, sw)
    in_maps = [
        {"x": x_sh[core], "w": w_t, "b": b_i8} for core in range(N_CORES)
    ]
    res = run_bass_kernel_spmd(
        nc, in_maps, core_ids=list(range(N_CORES)), trace=trace
    )
    out = np.concatenate(
        [res.results[core]["y"].reshape(IMGS_PER_CORE, COUT, H, W)
         for core in range(N_CORES)],
        axis=0,
    )
    return out, res


def kernel(x, weight, bias, step_x, step_w, step_b, shift_scale):
    out, _ = _run(x, weight, bias, step_x, step_w, step_b, shift_scale)
    return out


def kernel_profiled(x, weight, bias, step_x, step_w, step_b, shift_scale):
    return _run(x, weight, bias, step_x, step_w, step_b, shift_scale, trace=True)
